# revision 64
# baseline (speedup 1.0000x reference)
"""AttnDecoder kernel for 8 trn2 NeuronCores — latency-optimized chain design.

Math notes (exact in real arithmetic):
 - The reference's additive attention has no nonlinearity between W1/W2/w3, so
   softmax over s cancels every t-dependent term: attn (and ctx) are
   t-independent. ctx[b] is computed on the host.
 - logits = dec @ Wout[:, :H].T + (ctx @ Wout[:, H:].T + bout); the second
   term is t-independent and is added on the host.
 - Device work: the 2-layer LSTM recurrence (replicated on all 8 cores) and
   the dec-half of the vocab projection (vocab-sharded, 4096 padded cols per
   core).

Layout: everything is "output-transposed" — matmul outputs keep hidden/vocab
dims on partitions and the batch (8) on the free dim. Gate pre-activations for
step t live in one PSUM tile [128, 128] with col = gate*32 + j*8 + b
(j = h-dim block); one Sigmoid covers all four gates (tanh(z) = 2*sig(2z)-1
with the 2x baked into the g-gate weights/inputs).

Schedule: the two layers' recurrences are independent serial chains; layer 1
runs with an explicit 2-superstep lag so every one of its dependencies is at
least a superstep old and nothing stalls mid-stream on the engine queues
(engines execute any READY queued op when free, so a stalled op invites
overtaking but a long op can also delay a chain op that becomes ready during
it). Per superstep t:
  PE : rec0(t) | x1(t-2) | rec1(t-2) | injects | proj matmuls (last, so the
       proj PSUM result - and hence its copy - becomes ready late)
  Act: sig0(t) | sig1(t-2) | tanh0(t) | tanh1(t-2)
  DVE: c-update0(t) | c-update1(t-2) | hmul0(t) | proj PSUM->SBUF copy
       (WAW-gated behind hmul0 so it runs in the post-chain idle window)
  Pool: hmul1(t-2)  (all-SBUF; keeps it from head-of-line blocking DVE)
Recurrent weights are fp8-e4m3 (halves the weight-load DMA that gates the
layer-1 chain start; host-verified rel err ~6e-3); activations stay bf16.
"""

import numpy as np
import ml_dtypes

B, T, S = 8, 64, 128
V, E, H = 32000, 512, 512
NCORES = 8
VS = V // NCORES   # 4000 real vocab cols per core
VSP = 4096         # padded to 32 chunks of 128
NVC = VSP // 128   # 32 vocab chunks

_BF16 = ml_dtypes.bfloat16
_F8 = ml_dtypes.float8_e4m3fn
USE_FP8 = True

# Projection task list, in emission order: (vcg0, nvcg, t0, nt).
# Each task fills one PSUM tile with logits for vocab chunks
# [4*vcg0, 4*(vcg0+nvcg)) and decoder steps [t0, t0+nt), laid out
# col = vc_local*(nt*8) + tl*8 + b, then DMAs it straight to DRAM slot
# task_idx*512 (f32). Host unscrambles. Tail tasks pack several vocab
# groups so the end-of-program DMA burst stays short.
TASKS = (
    [(vcg, 1, 0, 16) for vcg in range(8)]
    + [(vcg, 1, 16, 16) for vcg in range(8)]
    + [(vcg, 1, 32, 16) for vcg in range(8)]
    + [(2 * g, 2, 48, 8) for g in range(4)]
    + [(4 * g, 4, 56, 4) for g in range(2)]
    + [(4 * g, 4, 60, 2) for g in range(2)]
    + [(0, 8, 62, 1), (0, 8, 63, 1)]
)
# earliest superstep at which each task's decT inputs exist (layer 1 runs
# 2 supersteps behind layer 0, so decT block s lands at superstep s+2)
TASK_AVAIL = ([18] * 8 + [34] * 8 + [50] * 8 + [58] * 4 + [62] * 2
              + [64] * 2 + [65, 66])


def _reorder_w(Wih, Whh):
    """[128, 8*2048]: rounds 0-3 = Wih K-chunks, 4-7 = Whh K-chunks.
    col j*512 + g*128 + x  <-  W[g*512 + 128j + x, 128*ki + p]; g-gate rows x2
    (tanh(z) = 2*sigmoid(2z) - 1 lets one Sigmoid call cover all gates)."""
    out = np.zeros((128, 8 * 2048), np.float32)
    for r in range(8):
        Wsrc = Wih if r < 4 else Whh
        ki = r % 4
        blk = Wsrc[:, 128 * ki:128 * (ki + 1)]          # [2048, 128] (gates, p)
        t_ = blk.reshape(4, 4, 128, 128)                # [g, j, x, p]
        t_ = t_.transpose(3, 1, 0, 2)                   # [p, j, g, x]
        out[:, r * 2048:(r + 1) * 2048] = t_.reshape(128, 2048)
    w5 = out.reshape(128, 8, 4, 4, 128)                 # [p, r, j, g, x]
    w5[:, :, :, 2, :] *= 2.0
    return out


def _build_nc():
    import concourse.bass as bass
    import concourse.bacc as bacc
    import concourse.mybir as mybir
    import concourse.tile as tile

    f32 = mybir.dt.float32
    bf16 = mybir.dt.bfloat16
    f8 = mybir.dt.float8e4 if USE_FP8 else mybir.dt.bfloat16
    AF = mybir.ActivationFunctionType
    OP = mybir.AluOpType

    nc = bacc.Bacc(None, target_bir_lowering=False)
    d = {}
    d["W0"] = nc.dram_tensor("W0", [128, 4 * 2048], f8, kind="ExternalInput")
    d["W1"] = nc.dram_tensor("W1", [128, 8 * 2048], f8, kind="ExternalInput")
    d["Wd"] = nc.dram_tensor("Wd", [128, 4 * VSP], bf16, kind="ExternalInput")
    d["ig0"] = nc.dram_tensor("ig0", [128, T * 128], bf16, kind="ExternalInput")
    d["misc"] = nc.dram_tensor("misc", [128, 512], bf16, kind="ExternalInput")
    out_d = nc.dram_tensor("out", [128, T * 256], bf16,
                           kind="ExternalOutput")

    with tile.TileContext(nc) as tc:
        with (
            tc.tile_pool(name="const", bufs=1) as cp,
            tc.tile_pool(name="work", bufs=64) as wp,
            tc.tile_pool(name="psA", bufs=2, space="PSUM") as ppA,
            tc.tile_pool(name="psB", bufs=2, space="PSUM") as ppB,
            tc.tile_pool(name="psP", bufs=4, space="PSUM") as ppP,
        ):
            W0s = cp.tile([128, 4 * 2048], f8, tag="W0s")
            W1s = cp.tile([128, 8 * 2048], f8, tag="W1s")
            Wds = cp.tile([128, 4 * VSP], bf16, tag="Wds")
            ig0s = cp.tile([128, T * 128], bf16, tag="ig0s")
            misc_sb = cp.tile([128, 512], bf16, tag="misc")
            ids = misc_sb[:, 0:128]
            b1s = misc_sb[:, 128:256]
            c_sb = misc_sb[:, 448:512]
            dec0T = cp.tile([128, (T + 1) * 32], bf16, tag="dec0T")
            decT = cp.tile([128, (T + 1) * 32], bf16, tag="decT")
            stage = cp.tile([128, T * 256], bf16, tag="stage")

            # Step-0 inputs first, then weights in consumption order; few
            # big DMAs (each dma_start costs ~625ns of serialized HWDGE).
            nc.sync.dma_start(misc_sb[:], d["misc"][:])
            for r in range(2):
                nc.sync.dma_start(W0s[:, r * 4096:(r + 1) * 4096],
                                  d["W0"][:, r * 4096:(r + 1) * 4096])
            nc.sync.dma_start(ig0s[:, 128:512], d["ig0"][:, 128:512])
            for r in range(2):
                nc.sync.dma_start(W1s[:, r * 8192:(r + 1) * 8192],
                                  d["W1"][:, r * 8192:(r + 1) * 8192])
            nc.sync.dma_start(ig0s[:, 512:2048], d["ig0"][:, 512:2048])
            for r in range(1, 4):
                nc.sync.dma_start(ig0s[:, r * 2048:(r + 1) * 2048],
                                  d["ig0"][:, r * 2048:(r + 1) * 2048])
            for r in range(4):
                nc.sync.dma_start(Wds[:, r * VSP:(r + 1) * VSP],
                                  d["Wd"][:, r * VSP:(r + 1) * VSP])

            own = [dec0T, decT]
            pools = [ppA, ppB]
            ps_t = [{}, {}]   # layer -> t -> psum tile
            sg_t = [{}, {}]   # layer -> t -> sigmoid output tile
            cn_t = [{}, {}]   # layer -> t -> new-c tile

            def inject(layer, t):
                ps = pools[layer].tile([128, 128], f32, tag=f"ps{layer}",
                                       name=f"ps{layer}_{t}")
                ps_t[layer][t] = ps
                if layer == 0:
                    src = (misc_sb[:, 320:448] if t == 0
                           else ig0s[:, 128 * t:128 * (t + 1)])
                else:
                    src = b1s[:]
                nc.tensor.matmul(ps[:], src, ids[:], start=True, stop=False,
                                 skip_group_check=True)

            def mm_x(s):
                # layer-1 input-side matmuls (dec0 -> gates); off the
                # critical path (dec0T[s+1] is ready before rec1(s) runs).
                ps = ps_t[1][s]
                for k in range(4):
                    hs = dec0T[:, 32 * (s + 1) + 8 * k: 32 * (s + 1) + 8 * k + 8]
                    for j in range(4):
                        for g in range(4):
                            mw = k * 2048 + (4 * j + g) * 128
                            nc.tensor.matmul(
                                ps[:, g * 32 + j * 8: g * 32 + j * 8 + 8],
                                W1s[:, mw: mw + 128],
                                hs, start=False, stop=False,
                                skip_group_check=True)

            def mm_rec(layer, t, gates):
                # gates: (0,1,2) for the c-path (i,f,g) or (3,) for o; the
                # sigmoid over cols 0:96 can then start after only 48 mms.
                ps = ps_t[layer][t]
                Wr = W0s if layer == 0 else W1s
                roff = 0 if layer == 0 else 4 * 2048
                if t == 0:
                    src = misc_sb
                    base = 256 + 32 * layer
                else:
                    src = own[layer]
                    base = 32 * t
                for k in range(4):
                    hs = src[:, base + 8 * k: base + 8 * k + 8]
                    for j in range(4):
                        for g in gates:
                            last = (k == 3 and j == 3 and g == gates[-1])
                            mw = roff + k * 2048 + (4 * j + g) * 128
                            nc.tensor.matmul(
                                ps[:, g * 32 + j * 8: g * 32 + j * 8 + 8],
                                Wr[:, mw: mw + 128],
                                hs, start=False, stop=last,
                                skip_group_check=True)

            def sig(layer, t, lo, hi):
                if lo == 0:
                    sg = wp.tile([128, 128], bf16, tag=f"sg{layer}")
                    sg_t[layer][t] = sg
                sg = sg_t[layer][t]
                nc.scalar.activation(sg[:, lo:hi], ps_t[layer][t][:, lo:hi],
                                     AF.Sigmoid)

            def cupd(layer, t):
                # c = sig(f)*c + sig(i)*tanh(zg); tanh(zg) = 2*sig(2zg)-1 and
                # the 2x is baked into the g-gate weights, so with
                # m2 = (sg'-0.5)*si:  c_new/2 = m2 + sig(f)*(c/2).
                sg = sg_t[layer][t]
                cs = c_sb[:, layer * 32:(layer + 1) * 32]
                m2 = wp.tile([128, 32], bf16, tag=f"m2{layer}")
                nc.vector.scalar_tensor_tensor(m2[:], sg[:, 64:96], 0.5,
                                               sg[:, 0:32],
                                               OP.subtract, OP.mult)
                m1 = wp.tile([128, 32], bf16, tag=f"m1{layer}")
                nc.vector.tensor_mul(m1[:], sg[:, 32:64], cs)
                nc.vector.tensor_add(cs, m2[:], m1[:])

            def ctanh(layer, t):
                cs = c_sb[:, layer * 32:(layer + 1) * 32]
                cn = wp.tile([128, 32], bf16, tag=f"cn{layer}")
                cn_t[layer][t] = cn
                nc.scalar.activation(cn[:], cs, AF.Tanh, scale=2.0)

            def hmul(layer, t):
                # layer 1's h-mul lands late in the superstep; on DVE it
                # head-of-line blocks layer 0's c-update, so it runs on the
                # otherwise-idle Pool engine (all-SBUF op).
                sg = sg_t[layer][t]
                eng = nc.vector if layer == 0 else nc.gpsimd
                eng.tensor_mul(own[layer][:, 32 * (t + 1):32 * (t + 2)],
                               sg[:, 96:128], cn_t[layer][t][:])

            # ---------- projection ----------
            # stage col = t*256 + vc*8 + b (t-major: tail regions DMA early)
            decv = decT.rearrange("p (s c) -> p s c", c=32)
            stg = stage.rearrange("p (t v b) -> p v t b", t=T, v=NVC, b=8)
            pq = list(TASKS)
            emit_proj_idx = [0]
            copy_q = []     # pending (psP, vcg0, nvcg, t0, nt, vl0, vl1)
            # DMA regions [t0, t1, copies_needed, copies_done]
            regions = [[0, 16, 8, 0], [16, 32, 8, 0], [32, 48, 8, 0],
                       [48, 56, 4, 0], [56, 60, 2, 0], [60, 62, 2, 0],
                       [62, 63, 1, 0], [63, 64, 1, 0]]

            def emit_proj(n):
                for _ in range(n):
                    if emit_proj_idx[0] >= len(pq):
                        return
                    i = emit_proj_idx[0]
                    emit_proj_idx[0] += 1
                    vcg0, nvcg, t0, nt = pq[i]
                    w = nt * 8
                    nvc = 4 * nvcg
                    psP = ppP.tile([128, 512], f32, tag="psP",
                                   name=f"psP_{t0}_{vcg0}")
                    nh = 2 if nt > 8 else 1     # split N to bound PE HOL delay
                    for vl in range(nvc):
                        vc = 4 * vcg0 + vl
                        for k in range(4):
                            for h2 in range(nh):
                                s0 = t0 + 1 + (nt // nh) * h2
                                sn = nt // nh
                                nc.tensor.matmul(
                                    psP[:, vl * w + sn * 8 * h2:
                                        vl * w + sn * 8 * (h2 + 1)],
                                    Wds[:, k * VSP + vc * 128:
                                        k * VSP + (vc + 1) * 128],
                                    decv[:, s0: s0 + sn, 8 * k:8 * k + 8],
                                    start=(k == 0 and h2 == 0),
                                    stop=(k == 3 and h2 == nh - 1),
                                    skip_group_check=True)
                    # copy emitted later at an engine-idle point
                    copy_q.append((psP, vcg0, nvcg, t0, nt, 0, nvc))

            def emit_copy(n, eng, gate=None):
                for _ in range(n):
                    if not copy_q:
                        return
                    psP, vcg0, nvcg, t0, nt, vl0, vl1 = copy_q.pop(0)
                    pv4 = psP.rearrange("p (v t b) -> p v t b", t=nt, b=8)
                    src = pv4[:, vl0:vl1, :, :]
                    dst = stg[:, 4 * vcg0 + vl0:4 * vcg0 + vl1, t0:t0 + nt, :]
                    if gate is not None:
                        # 1-col dummy write into dst (overwritten below):
                        # WAW-orders the copy after `gate`, so the big copy
                        # only becomes ready in the post-chain idle window
                        # instead of grabbing the engine mid-chain.
                        nc.vector.tensor_copy(
                            stage[:, t0 * 256 + (4 * vcg0 + vl0) * 8:
                                  t0 * 256 + (4 * vcg0 + vl0) * 8 + 1],
                            gate[:, 0:1])
                    if eng == "act":
                        nc.scalar.activation(dst, src, AF.Copy)
                    else:
                        nc.vector.tensor_copy(dst, src)
                    for reg in regions:
                        if reg[0] <= t0 < reg[1]:
                            reg[3] += 1
                            if reg[3] == reg[2]:
                                nc.sync.dma_start(
                                    out_d[:, reg[0] * 256:reg[1] * 256],
                                    stage[:, reg[0] * 256:reg[1] * 256])

            # ---------- main loop ----------
            # Layer 1 runs with an explicit 2-superstep lag: by the time its
            # ops are issued, every dependency (hm1(s-1), dec0T[s+1]) is from
            # >=1 superstep ago, so nothing stalls mid-stream on the in-order
            # engine queues.
            inject(0, 0)
            for t in range(T):
                s = t - 2   # layer-1 step handled this superstep
                mm_rec(0, t, (0, 1, 2))
                mm_rec(0, t, (3,))
                sig(0, t, 0, 128)
                if s >= 0:
                    mm_x(s)
                    mm_rec(1, s, (0, 1, 2))
                    mm_rec(1, s, (3,))
                    sig(1, s, 0, 128)
                # proj matmuls near the end of the PE stream: psP completes
                # ~2us into the superstep, so its copy becomes READY only
                # in the post-chain DVE idle window (engines run any ready
                # op when free, so early-ready copies would collide with
                # the chain's c-update instead).
                if (emit_proj_idx[0] < len(pq)
                        and TASK_AVAIL[emit_proj_idx[0]] <= t):
                    emit_proj(1)
                if t + 1 < T:
                    inject(0, t + 1)
                if t >= 1:
                    inject(1, t - 1)
                cupd(0, t)
                ctanh(0, t)
                hmul(0, t)
                if s >= 0:
                    cupd(1, s)
                    ctanh(1, s)
                    hmul(1, s)
                emit_copy(1, "dve", gate=dec0T[:, 32 * (t + 1):32 * (t + 1) + 1])
            # drain layer-1 steps T-2, T-1
            for s in (T - 2, T - 1):
                mm_x(s)
                mm_rec(1, s, (0, 1, 2))
                mm_rec(1, s, (3,))
                sig(1, s, 0, 128)
                if s == T - 2:
                    inject(1, T - 1)
                    emit_proj(2)    # (60,2)x2: decT[61..62] exist by now
                else:
                    emit_proj(1)    # (62,1): decT[63] exists by now
                cupd(1, s)
                ctanh(1, s)
                hmul(1, s)
                # copies gated behind this drain step's hmul so they can't
                # grab DVE/Act mid-chain
                g_ap = decT[:, 32 * (s + 1):32 * (s + 1) + 1]
                emit_copy(1, "dve", gate=g_ap)
                emit_copy(1, "act", gate=g_ap)
            emit_proj(len(pq) - emit_proj_idx[0])
            while copy_q:
                emit_copy(1, "dve")
                emit_copy(1, "act")
    nc.finalize()
    return nc


_NC_CACHE = None


def _get_nc():
    global _NC_CACHE
    if _NC_CACHE is None:
        _NC_CACHE = _build_nc()
    return _NC_CACHE


def _host_inputs(input_ids, enc_output, h0, c0, emb, Wih0, Whh0, bih0, bhh0,
                 Wih1, Whh1, bih1, bhh1, W1, b1, W2, b2, w3, b3, Wout, bout):
    f32 = np.float32
    x = np.asarray(emb, f32)[np.asarray(input_ids).astype(np.int64)]  # [B,T,E]

    # Layer-0 input projection on the host (exact), g-gate x2, bias folded in.
    ig0 = x @ np.asarray(Wih0, f32).T + (np.asarray(bih0, f32)
                                         + np.asarray(bhh0, f32))   # [B,T,2048]
    ig0 = ig0.reshape(B, T, 4, 4, 128)          # [b,t,g,j,x]
    ig0[:, :, 2] *= 2.0
    ig0T = ig0.transpose(2, 3, 0, 1, 4).reshape(128, T * 128)  # [(g,j,b),(t,x)]

    b1v = (np.asarray(bih1, f32) + np.asarray(bhh1, f32)).reshape(4, 4, 128)
    b1v = b1v.copy()
    b1v[2] *= 2.0                               # [g,j,x]
    b1T = np.broadcast_to(b1v[:, :, None, :], (4, 4, 8, 128)).reshape(128, 128)

    def h0T(hl):
        return hl.T.reshape(4, 128, 8).transpose(1, 0, 2).reshape(128, 32)

    c0a = (np.asarray(c0, f32) * 0.5).reshape(2, 8, 4, 128)
    c0T = c0a.transpose(3, 0, 2, 1).reshape(128, 64)  # [x, (layer,j,b)]

    # collapsed attention (exact in real arithmetic; see module docstring)
    u = np.asarray(W2, f32).T @ np.asarray(w3, f32)[0]
    ue = np.asarray(W1, f32)[:, :H].T @ u
    sc = np.asarray(enc_output, f32) @ ue                  # [B,S]
    sc = sc - sc.max(-1, keepdims=True)
    a = np.exp(sc)
    a /= a.sum(-1, keepdims=True)
    ctxh = np.einsum('bs,bsh->bh', a, np.asarray(enc_output, f32))  # [B,H]

    Wo_full = np.asarray(Wout, f32)                        # [V, 2H]
    bo_full = np.asarray(bout, f32)
    # t-independent half of the projection, added on the host
    ctxadd = ctxh @ Wo_full[:, H:].T + bo_full             # [B, V]

    Wrec = _reorder_w(np.asarray(Wih0, f32), np.asarray(Whh0, f32))
    misc = np.concatenate([np.eye(128, dtype=f32), b1T,
                           h0T(np.asarray(h0, f32)[0]),
                           h0T(np.asarray(h0, f32)[1]),
                           ig0T[:, 0:128], c0T], axis=1)
    base = {
        "W0": np.ascontiguousarray(Wrec[:, 4 * 2048:]).astype(_F8 if USE_FP8 else _BF16),
        "W1": _reorder_w(np.asarray(Wih1, f32),
                         np.asarray(Whh1, f32)).astype(_F8 if USE_FP8 else _BF16),
        "ig0": ig0T.astype(_BF16),
        "misc": misc.astype(_BF16),
    }
    maps = []
    for k in range(NCORES):
        lo = k * VS
        sh = np.zeros((VSP, H), f32)
        n = min(VSP, V - lo)
        sh[:n] = Wo_full[lo:lo + n, :H]
        t_ = sh.reshape(NVC, 128, 4, 128).transpose(3, 2, 0, 1)  # [p,k,vc,m]
        m = dict(base)
        m["Wd"] = np.ascontiguousarray(t_.reshape(128, 4 * VSP)).astype(_BF16)
        maps.append(m)
    return maps, ctxadd


def kernel(**inputs):
    from concourse.bass_utils import run_bass_kernel_spmd
    nc = _get_nc()
    maps, ctxadd = _host_inputs(**inputs)
    res = run_bass_kernel_spmd(nc, maps, list(range(NCORES))).results
    full = np.zeros((B, T, V), np.float32)
    for k in range(NCORES):
        o = np.asarray(res[k]["out"], np.float32)   # [128, T*256]
        o = o.reshape(128, T, NVC, 8)               # [x, t, vc, b]
        o = o.transpose(3, 1, 2, 0).reshape(B, T, VSP)
        n = min(VS, V - k * VS)
        full[:, :, k * VS:k * VS + n] = o[:, :, :n]
    full += ctxadd[:, None, :]
    return full
